# revision 1
# baseline (speedup 1.0000x reference)
"""Trainium2 Bass kernel for nn_BondDecoder (topk_masking).

Strategy (see test.py for the dev harness):
  - Data-parallel over batch: 64 batches -> 8 cores x 8 slots.
  - Rows/columns are compacted to the unmasked (src_mask == False) positions
    per batch: only those positions contribute to the loss, so all O(L^2)
    work shrinks by ~4x.  Batches are sorted by compact size and dealt so
    that slot k has a similar size on every core (SPMD shares one program).
  - The two back-to-back linear layers (pointwise conv -> in_proj) are
    composed into a single weight on the host; biases are folded in via an
    all-ones row appended to x^T (a K=1 accumulation matmul).
  - scores_h = q_h k_h^T via PE (f32r, 1 cyc/row); a K=1 matmul accumulates
    -100 * padmark_j to kill padded columns before exp.
  - exp + per-row sums in ONE ScalarE pass (activation accum_out).
  - u = sum_h r_h E_h^inc - r_h E_h^dec via fused scalar_tensor_tensor,
    with r = valid/rowsum (division folded into per-partition scalars; the
    4*mean-over-heads factor cancels exactly).
  - bond count maps built on-device with iota + is_equal compare-accumulate.
  - loss_b = sum_ij (1 - t_i t_j) m^2 via scalar_tensor_tensor accum_out row
    sums and a final ones-vector matmul over partitions.

The host side only does layout/index preprocessing: sharding, gather of
unmasked columns, weight composition, bond index remapping.
"""

import os
import sys
from contextlib import ExitStack

if "/opt/trn_rl_repo" not in sys.path:
    sys.path.insert(0, "/opt/trn_rl_repo")

import numpy as np

import concourse.bacc as bacc
import concourse.bass as bass
import concourse.tile as tile
from concourse import bass_utils, mybir

L, B, DIM = 512, 64, 256
H, HD, MB = 4, 64, 6
NCORES = 8
BPC = B // NCORES  # slots per core

F32 = mybir.dt.float32
F32R = mybir.dt.float32r
EDT = mybir.dt.float16  # dtype for exp/count maps (flip to float32 for accuracy)
NP_EDT = np.float16

_CACHE = {}


def _chunks(n):
    out = []
    i = 0
    while i < n:
        out.append((i, min(128, n - i)))
        i += 128
    return out


def _build_program(slot_n, nrcap, totc):
    nc = bacc.Bacc(
        "TRN2",
        target_bir_lowering=False,
        debug=False,
        enable_asserts=False,
        num_devices=NCORES,
    )
    dt = mybir.dt
    xT_d = nc.dram_tensor("xT", [DIM + 1, totc], F32R, kind="ExternalInput").ap()
    wall_d = nc.dram_tensor("wall", [4, DIM + 1, DIM], F32R, kind="ExternalInput").ap()
    mrow_d = nc.dram_tensor("mrow", [1, totc], F32R, kind="ExternalInput").ap()
    trow_d = nc.dram_tensor("trow", [1, totc], EDT, kind="ExternalInput").ap()
    trow2_d = nc.dram_tensor("trow2", [1, totc], EDT, kind="ExternalInput").ap()
    rowdat_d = nc.dram_tensor(
        "rowdat", [BPC, nrcap, 16], F32, kind="ExternalInput"
    ).ap()
    loss_d = nc.dram_tensor("loss", [1, BPC], F32, kind="ExternalOutput").ap()

    nmax = max(slot_n)
    AL = mybir.AluOpType

    with ExitStack() as ctx:
        tc = ctx.enter_context(tile.TileContext(nc))
        singles = ctx.enter_context(tc.tile_pool(name="singles", bufs=1))
        xpool = ctx.enter_context(tc.tile_pool(name="xpool", bufs=3))
        qk = ctx.enter_context(tc.tile_pool(name="qk", bufs=3))
        epool = ctx.enter_context(tc.tile_pool(name="epool", bufs=4))
        work = ctx.enter_context(tc.tile_pool(name="work", bufs=4))
        small = ctx.enter_context(tc.tile_pool(name="small", bufs=8))
        ppool = ctx.enter_context(tc.tile_pool(name="pp", bufs=1, space="PSUM"))
        spool = ctx.enter_context(tc.tile_pool(name="sp", bufs=5, space="PSUM"))
        fpool = ctx.enter_context(tc.tile_pool(name="fp", bufs=1, space="PSUM"))

        # ---- constants / weights (once per core) ----
        w0, w1, wb = [], [], []
        for p in range(4):
            t0 = singles.tile([128, DIM], F32R, tag=f"w0_{p}")
            nc.sync.dma_start(out=t0, in_=wall_d[p, 0:128, :])
            t1 = singles.tile([128, DIM], F32R, tag=f"w1_{p}")
            nc.sync.dma_start(out=t1, in_=wall_d[p, 128:256, :])
            t2 = singles.tile([1, DIM], F32R, tag=f"wb_{p}")
            nc.sync.dma_start(out=t2, in_=wall_d[p, 256:257, :])
            w0.append(t0)
            w1.append(t1)
            wb.append(t2)

        ones128 = singles.tile([128, 1], F32, tag="ones128")
        nc.vector.memset(ones128, 1.0)
        res = singles.tile([128, BPC * 3], F32, tag="res")
        nc.vector.memset(res, 0.0)

        ci_i = singles.tile([128, nmax], dt.int32, tag="ci_i")
        nc.gpsimd.iota(ci_i, pattern=[[1, nmax]], base=0, channel_multiplier=0)
        colidx = singles.tile([128, nmax], EDT, tag="colidx")
        nc.vector.tensor_copy(out=colidx, in_=ci_i)

        def bcast(pool_tag, src_row, n):
            # DMA the [1, n] DRAM row in, then broadcast across partitions
            row = small.tile([1, nmax], EDT, tag=pool_tag + "_row")
            nc.sync.dma_start(out=row[0:1, 0:n], in_=src_row)
            dst = xpool.tile([128, n], EDT, tag=pool_tag)
            nc.gpsimd.partition_broadcast(dst, row[0:1, 0:n])
            return dst

        off = 0
        for s, n in enumerate(slot_n):
            chs = _chunks(n)
            # ---- load x^T (augmented with ones row) ----
            xall = xpool.tile([128, 2, n], F32R, tag="x0")
            nc.sync.dma_start(
                out=xall,
                in_=xT_d[0:256, off : off + n].rearrange("(a p) c -> p a c", p=128),
            )
            x0 = xall[:, 0, :]
            x1 = xall[:, 1, :]
            xb = xpool.tile([1, n], F32R, tag="xb")
            nc.sync.dma_start(out=xb, in_=xT_d[256:257, off : off + n])
            mrow_t = xpool.tile([1, n], F32R, tag="mr")
            nc.sync.dma_start(out=mrow_t, in_=mrow_d[0:1, off : off + n])
            tbc = bcast("tbc", trow_d[0:1, off : off + n], n)
            t2bc = bcast("t2bc", trow2_d[0:1, off : off + n], n)

            # ---- projections: psum [128, 2, 512] per (branch, head-pair) ----
            # wall order: 0=inc_q 1=inc_k 2=dec_q 3=dec_k
            qkt = {}
            for br in range(2):
                for g in range(2):  # head-pair (out-channel 128-block)
                    pp = ppool.tile([128, 2, 512], F32, tag="pp")
                    for j, p in enumerate((2 * br, 2 * br + 1)):
                        mg = slice(128 * g, 128 * g + 128)
                        nc.tensor.matmul(
                            pp[:, j, 0:n],
                            w0[p][:, mg],
                            x0,
                            start=True,
                            stop=False,
                        )
                        nc.tensor.matmul(
                            pp[:, j, 0:n],
                            w1[p][:, mg],
                            x1,
                            start=False,
                            stop=False,
                        )
                        nc.tensor.matmul(
                            pp[:, j, 0:n],
                            wb[p][:, mg],
                            xb[:],
                            start=False,
                            stop=True,
                        )
                    qt = qk.tile([128, 2, n], F32R, tag=f"qk{br}{g}")
                    nc.vector.tensor_copy(out=qt, in_=pp[:, :, 0:n])
                    qkt[(br, g)] = qt

            rd_all = small.tile([128, 3, 16], F32, tag="rd")
            nc.sync.dma_start(
                out=rd_all,
                in_=rowdat_d[s].rearrange("(c p) f -> p c f", p=128),
            )

            # ---- per row-chunk: scores -> exp -> combine -> counts -> reduce
            for ic, (i0, il) in enumerate(chs):
                rd = rd_all[:, ic, :]
                Sall = small.tile([128, 8], F32, tag="Sall")
                Es = []
                for m in range(8):
                    br, h = m // 4, m % 4
                    g, sub = h // 2, h % 2
                    rows = slice(64 * sub, 64 * sub + 64)
                    ps = spool.tile([128, 512], F32, tag="sc")
                    nc.tensor.matmul(
                        ps[0:il, 0:n],
                        qkt[(br, g)][rows, 0, i0 : i0 + il],
                        qkt[(br, g)][rows, 1, 0:n],
                        start=True,
                        stop=False,
                    )
                    nc.tensor.matmul(
                        ps[0:il, 0:n],
                        xb[0:1, i0 : i0 + il],
                        mrow_t[0:1, 0:n],
                        start=False,
                        stop=True,
                    )
                    e = epool.tile([128, n], EDT, tag=f"E{m}")
                    nc.scalar.activation(
                        out=e[0:il, :],
                        in_=ps[0:il, 0:n],
                        func=mybir.ActivationFunctionType.Exp,
                        accum_out=Sall[0:il, m : m + 1],
                    )
                    Es.append(e)

                # r = valid / S  (S > 0 always: padded rows sum ~n)
                r = small.tile([128, 8], F32, tag="r")
                nc.vector.reciprocal(out=r[0:il], in_=Sall[0:il])
                nc.vector.tensor_scalar_mul(
                    out=r[0:il], in0=r[0:il], scalar1=rd[0:il, 14:15]
                )

                # u = sum_h r_h E_h^inc - sum_h r_h E_h^dec
                # inc branch: fused STT chain on DVE; dec branch: GPSIMD
                # (scalar_tensor_tensor is DVE-only on this HW) in parallel.
                u = work.tile([128, n], EDT, tag="u")
                nc.vector.tensor_scalar_mul(
                    out=u[0:il], in0=Es[0][0:il], scalar1=r[0:il, 0:1]
                )
                for m in range(1, 4):
                    nc.vector.scalar_tensor_tensor(
                        out=u[0:il],
                        in0=Es[m][0:il],
                        scalar=r[0:il, m : m + 1],
                        in1=u[0:il],
                        op0=AL.mult,
                        op1=AL.add,
                    )
                ud = work.tile([128, n], EDT, tag="ud")
                nc.gpsimd.tensor_scalar_mul(
                    out=ud[0:il], in0=Es[4][0:il], scalar1=r[0:il, 4:5]
                )
                for m in range(5, 8):
                    udt = work.tile([128, n], EDT, tag="udt")
                    nc.gpsimd.tensor_scalar_mul(
                        out=udt[0:il], in0=Es[m][0:il], scalar1=r[0:il, m : m + 1]
                    )
                    nc.gpsimd.tensor_add(ud[0:il], ud[0:il], udt[0:il])
                nc.vector.tensor_tensor(
                    out=u[0:il], in0=u[0:il], in1=ud[0:il], op=AL.subtract
                )

                # tgt bond counts (weighted by (1-t_j) then -(1-t_i))
                ct = work.tile([128, n], EDT, tag="ct")
                nc.vector.tensor_scalar(
                    out=ct[0:il],
                    in0=colidx[0:il, 0:n],
                    scalar1=rd[0:il, 6:7],
                    scalar2=None,
                    op0=AL.is_equal,
                )
                for m in range(1, 6):
                    nc.vector.scalar_tensor_tensor(
                        out=ct[0:il],
                        in0=colidx[0:il, 0:n],
                        scalar=rd[0:il, 6 + m : 7 + m],
                        in1=ct[0:il],
                        op0=AL.is_equal,
                        op1=AL.add,
                    )
                nc.vector.tensor_mul(ct[0:il], ct[0:il], t2bc[0:il])
                # src bond counts accumulate straight onto u
                for m in range(6):
                    nc.vector.scalar_tensor_tensor(
                        out=u[0:il],
                        in0=colidx[0:il, 0:n],
                        scalar=rd[0:il, m : m + 1],
                        in1=u[0:il],
                        op0=AL.is_equal,
                        op1=AL.add,
                    )
                # m = u + cs - (1-t_i)(1-t_j) ct
                nc.vector.scalar_tensor_tensor(
                    out=u[0:il],
                    in0=ct[0:il],
                    scalar=rd[0:il, 13:14],
                    in1=u[0:il],
                    op0=AL.mult,
                    op1=AL.add,
                )

                # loss rows: R - t_i * T with R = sum_j m^2, T = sum_j t_j m^2
                # (the R pass materializes m^2 in scr; the T pass reuses it)
                scr = work.tile([128, n], EDT, tag="scr")
                scr2 = work.tile([128, n], EDT, tag="scr2")
                Racc = small.tile([128, 1], F32, tag="Racc")
                Tacc = small.tile([128, 1], F32, tag="Tacc")
                nc.vector.scalar_tensor_tensor(
                    out=scr[0:il],
                    in0=u[0:il],
                    scalar=1.0,
                    in1=u[0:il],
                    op0=AL.mult,
                    op1=AL.mult,
                    accum_out=Racc[0:il],
                )
                nc.vector.scalar_tensor_tensor(
                    out=scr2[0:il],
                    in0=scr[0:il],
                    scalar=1.0,
                    in1=tbc[0:il],
                    op0=AL.mult,
                    op1=AL.mult,
                    accum_out=Tacc[0:il],
                )
                col = s * 3 + ic
                nc.vector.scalar_tensor_tensor(
                    out=res[0:il, col : col + 1],
                    in0=Tacc[0:il],
                    scalar=rd[0:il, 12:13],
                    in1=Racc[0:il],
                    op0=AL.mult,
                    op1=AL.add,
                )
            off += n

        # ---- final: column sums over partitions, then fold 3 chunks/slot ----
        pfin = fpool.tile([1, BPC, 3], F32, tag="fin")
        nc.tensor.matmul(pfin[:, :, :], ones128, res, start=True, stop=True)
        fin_sb = singles.tile([1, BPC, 3], F32, tag="fin_sb")
        nc.vector.tensor_copy(out=fin_sb, in_=pfin)
        lt = singles.tile([1, BPC], F32, tag="lt")
        nc.vector.tensor_add(lt, fin_sb[:, :, 0], fin_sb[:, :, 1])
        nc.vector.tensor_add(lt, lt, fin_sb[:, :, 2])
        nc.sync.dma_start(out=loss_d, in_=lt)

    nc.compile()
    return nc


def _prep(inputs):
    me = np.ascontiguousarray(np.asarray(inputs["molecule_embedding"], np.float32))
    src_mask = np.asarray(inputs["src_mask"]).astype(bool)
    tgt_mask = np.asarray(inputs["tgt_mask"]).astype(bool)
    src_bond = np.asarray(inputs["src_bond"]).astype(np.int64)
    tgt_bond = np.asarray(inputs["tgt_bond"]).astype(np.int64)

    # weight composition: (pointwise conv then in_proj) == single linear
    def f32(k):
        return np.asarray(inputs[k], np.float64)

    WQ = {}
    for br, pre in ((0, "inc"), (1, "dec")):
        for qk_, (w2, b2, w1, b1) in (
            ("q", (f32(f"{pre}_wq"), f32(f"{pre}_bq"), f32(f"{pre}_q_w"), f32(f"{pre}_q_b"))),
            ("k", (f32(f"{pre}_wk"), f32(f"{pre}_bk"), f32(f"{pre}_k_w"), f32(f"{pre}_k_b"))),
        ):
            W = w2 @ w1
            bvec = w2 @ b1 + b2
            if qk_ == "q":
                W = W * (HD ** -0.5)
                bvec = bvec * (HD ** -0.5)
            WQ[(br, qk_)] = (W.astype(np.float32), bvec.astype(np.float32))

    wall = np.zeros((4, DIM + 1, DIM), np.float32)
    for p, (br, qk_) in enumerate(((0, "q"), (0, "k"), (1, "q"), (1, "k"))):
        W, bvec = WQ[(br, qk_)]
        wall[p, 0:DIM, :] = W.T  # [d, e]
        wall[p, DIM, :] = bvec

    kept = [np.nonzero(~src_mask[b])[0] for b in range(B)]
    nk = np.array([len(k) for k in kept])
    order = np.argsort(nk, kind="stable")
    slot_n = []
    for s in range(BPC):
        mx = nk[order[s * NCORES : (s + 1) * NCORES]].max()
        slot_n.append(int(-(-mx // 8) * 8))
    totc = int(sum(slot_n))
    nrcap = 384

    in_maps = []
    for c in range(NCORES):
        xT = np.zeros((DIM + 1, totc), np.float32)
        xT[DIM, :] = 1.0
        mrow = np.zeros((1, totc), np.float32)
        trow = np.ones((1, totc), NP_EDT)
        trow2 = np.zeros((1, totc), NP_EDT)
        rowdat = np.zeros((BPC, nrcap, 16), np.float32)
        rowdat[:, :, 0:12] = -1.0
        rowdat[:, :, 12] = -1.0  # -t_i for padded rows (t=1)

        off = 0
        for s in range(BPC):
            n = slot_n[s]
            b = int(order[s * NCORES + c])
            kb = kept[b]
            m = len(kb)
            xT[0:DIM, off : off + m] = me[kb, b, :].T
            mrow[0, off + m : off + n] = -100.0
            tb = tgt_mask[b, kb].astype(NP_EDT)
            trow[0, off : off + m] = tb
            trow2[0, off : off + m] = 1.0 - tb
            remap = np.full(L, -1.0, np.float32)
            remap[kb] = np.arange(m, dtype=np.float32)
            rowdat[s, 0:m, 0:6] = remap[src_bond[b, kb, :]]
            rowdat[s, 0:m, 6:12] = remap[tgt_bond[b, kb, :]]
            rowdat[s, 0:m, 12] = -tb
            rowdat[s, 0:m, 13] = -(1.0 - tb)
            rowdat[s, 0:m, 14] = 1.0
            off += n

        in_maps.append(
            {
                "xT": xT,
                "wall": wall,
                "mrow": mrow,
                "trow": trow,
                "trow2": trow2,
                "rowdat": rowdat,
            }
        )
    return in_maps, tuple(slot_n), nrcap, totc, order


def kernel(**inputs) -> np.ndarray:
    in_maps, slot_n, nrcap, totc, order = _prep(inputs)
    key = (slot_n, nrcap, totc, str(EDT))
    if key not in _CACHE:
        _CACHE[key] = _build_program(list(slot_n), nrcap, totc)
    nc = _CACHE[key]
    res = bass_utils.run_bass_kernel_spmd(
        nc,
        in_maps,
        core_ids=list(range(NCORES)),
        trace=False,
    )
    global LAST_RESULTS
    LAST_RESULTS = res
    loss = np.zeros(B, np.float32)
    for c in range(NCORES):
        per_core = res.results[c]["loss"].reshape(BPC)
        for s in range(BPC):
            loss[order[s * NCORES + c]] = per_core[s]
    return loss


LAST_RESULTS = None

